# revision 23
# baseline (speedup 1.0000x reference)
"""ActiveShiftLayer Trainium2 kernel (fp16, 2-tile interleaved pipeline).

Measured on trn2 (8 cores): ~68.6-69.9 us HW exec (vs 84-88 us for the
f32 v1 baseline), l2 rel err ~4.3e-4. Span budget: ~11 us head (7 us
runtime preamble gates the first DMA issue; +~2 us DMA completion-sem
receipt latency) + ~52 us TensorE-paced compute (~87% dense, ~45 us
busy) + ~6 us framework semaphore teardown. TensorE is the pacer;
Vector/Scalar run ~40 us each. Engine-assignment patterns are tunable
via ASL_HPAT_B0/B1 ('p'/'d' per piece pair) and ASL_C2PAT ('s'/'v');
shifting work off TensorE was measured a wash (+-1.5 us run variance).

out[n,c,h,w] = bilinear sample of x[n,c, h+alpha_c, w+beta_c], zero outside
the spatial extent.

alpha,beta in [-1,1) => floor in {-1,0}, so the bilinear sample is a
separable 3-tap convolution along H then W; the two outer taps are
EXCLUSIVE per channel (floor -1 -> only the -1 tap, floor 0 -> only the +1
tap), so each stage is a per-channel 2-tap.

Sharding: data-parallel over batch (N=32 -> 4 per core) x 2 channel blocks
of 128. Channels are HOST-SORTED by the sign of the W-shift so blocks are
(nearly) side-pure -> fewer side passes; host un-permutes the output.

Pipeline (all fp16; input staged fp16 on host -> halves HBM reads; total
quantization ~4e-4 vs the 2e-2 gate):
- V-stage on TensorE: per 512-chunk, accumulating diag-matmul taps at row
  offsets -56/0/+56 into the guarded X -> PSUM f32 (quarter-plane pieces,
  2-bank PSUM tiles, 4-deep ring).
- copy1 on ScalarE: PSUM -> A[1+i] = vt[i] fp16 (odd element offset so the
  flat +-1 taps of the PE H-stage read 4B-aligned... and cheap).
- H-stage split per piece-pair between:
  * TensorE ('p'): center + per-need side diag matmuls at flat offsets
    into A -> PSUM, drained by copy2 (ScalarE/VectorE per pattern); flat
    side taps wrap at row edges -> per-piece strided fixup STTs (VectorE,
    negated weights).
  * VectorE ('d'): tensor_scalar center + per-need side STTs with 2D
    access patterns that EXCLUDE the wrapped column (no fixups needed),
    batched over piece pairs.
- SWDGE (GpSimd) stores the fp16 tile; sync HWDGE carries only loads.

TWO TILES are emitted interleaved (independent dependency chains) so each
engine always has runnable work during the other chain's cross-engine
latency hops, and TensorE stays busy enough for HAM to hold the 2.4 GHz
clock.
"""

import os
import numpy as np

N, C, H, W = 32, 256, 56, 56
NCORES = 8
NSH = N // NCORES  # batches per core
P = 128
CB = C // P        # channel blocks
HW = H * W         # 3136
CHUNK = int(os.environ.get("ASL_CHUNK", "512"))
XLEN = W + HW + W + 16  # guard row + plane + guard row + pad (fp16 elems)
ALEN = HW + 2      # vt at offset 1 with zero guards at 0 and HW+1
PSLEN = 1024       # 2 PSUM banks; pieces are <= 784 elems

# H-stage engine pattern per channel block, cycled over piece PAIRS:
# 'p' = TensorE, 'd' = VectorE
HPAT = (os.environ.get("ASL_HPAT_B0", "dpdpd"),
        os.environ.get("ASL_HPAT_B1", "dpdd"))
# copy2 engine pattern over PE-H pieces: 's' = ScalarE, 'v' = VectorE
C2PAT = os.environ.get("ASL_C2PAT", "ssv")

_CACHE = {}


def _build_nc(flags):
    # flags[cb] = (needL, needR, needVm, needVp)
    import concourse.bacc as bacc
    import concourse.mybir as mybir
    import concourse.tile as tile

    f16 = mybir.dt.float16
    f32 = mybir.dt.float32
    mult = mybir.AluOpType.mult
    add = mybir.AluOpType.add
    act_copy = mybir.ActivationFunctionType.Copy

    nc = bacc.Bacc()
    xs = nc.dram_tensor("xs", [NSH, C, H, W], f16, kind="ExternalInput")
    # wd[cb] = [diag(wv_m1)|diag(wv_0)|diag(wv_p1)|diag(wh_m1)|diag(wh_0)|
    #           diag(wh_p1)] for (sorted) channel block cb
    wd = nc.dram_tensor("wd", [CB, P, 6 * P], f16, kind="ExternalInput")
    # wv[cb] columns: [wh_m1, wh_0, wh_p1, -wh_m1, -wh_p1]
    wv = nc.dram_tensor("wv", [CB, P, 5], f32, kind="ExternalInput")
    ys = nc.dram_tensor("ys", [NSH, C, H, W], f16, kind="ExternalOutput")

    with tile.TileContext(nc) as tc:
        with tc.tile_pool(name="wp", bufs=1) as wp, \
             tc.tile_pool(name="xp", bufs=5) as xpool, \
             tc.tile_pool(name="ap", bufs=4) as apool, \
             tc.tile_pool(name="op", bufs=4) as opool, \
             tc.tile_pool(name="ps0", bufs=2, space="PSUM") as ppool0, \
             tc.tile_pool(name="ps1", bufs=2, space="PSUM") as ppool1:
            ppools = (ppool0, ppool1)

            wdt = []
            wvt = []

            # dummy activation: hoists the lazy ACT_TABLE_LOAD (~1.3us)
            # into the runtime preamble, off the first copy1's critical path
            sc = wp.tile([P, 2], f32, tag="sc")
            nc.gpsimd.memset(sc[:], 0.0)
            nc.scalar.activation(sc[:], sc[:],
                                 mybir.ActivationFunctionType.Copy)
            # PE warm-up: ~3us of dummy matmuls on a zeroed scratch tile
            # during the (PE-idle) load head flips the HAM clock gate to
            # 2.4 GHz before the first real matmul
            wu = wp.tile([P, 2 * P], f16, tag="wu")
            nc.gpsimd.memset(wu[:].bitcast(f32), 0.0)
            PSW = ppool0.tile([P, PSLEN], f32, tag="ps")
            for _ in range(14):
                # N=256 cold MMs ~213ns each -> ~3us of continuous PE busy,
                # ending roughly when the first real tile data has landed
                nc.tensor.matmul(PSW[:, 0:256], wu[:, 0:P], wu[:, 0:2 * P],
                                 start=True, stop=True)

            def load_wd(cb):
                t = wp.tile([P, 6 * P], f16, tag=f"wd{cb}")
                nc.sync.dma_start(t[:], wd[cb])
                wdt.append(t)

            def load_wv(cb):
                v = wp.tile([P, 5], f32, tag=f"wv{cb}")
                nc.sync.dma_start(v[:], wv[cb])
                wvt.append(v)

            tiles = [(n, cb) for n in range(NSH) for cb in range(CB)]
            NT = len(tiles)

            def bounds(idx):
                if idx == 0:
                    return [0, 4, 9, 16, 28, 42, 56]
                if idx == NT - 1:
                    return [0, 14, 28, 42, 49, 56]
                return [0, 14, 28, 42, 56]

            xtiles = {}

            def issue_load(idx):
                ln, lcb = tiles[idx]
                lcs = slice(lcb * P, (lcb + 1) * P)
                X = xpool.tile([P, XLEN], f16, tag="X")
                nc.gpsimd.memset(X[:, 0:W].bitcast(f32), 0.0)
                nc.gpsimd.memset(X[:, W + HW:W + HW + W].bitcast(f32), 0.0)
                xflat = xs[ln, lcs, :, :].rearrange("p h w -> p (h w)")
                b = bounds(idx)
                if idx < 2:
                    # fine segments on the pipeline-fill tiles so the first
                    # matmuls can start as soon as a few rows have landed
                    cuts = [min(r + 1, H) for r in b[1:-1]] + [H]
                else:
                    cuts = [min(b[len(b) // 2] + 1, H), H]
                r0 = 0
                for r1 in cuts:
                    if r1 > r0:
                        nc.sync.dma_start(X[:, W + r0 * W:W + r1 * W],
                                          xflat[:, r0 * W:r1 * W])
                    r0 = r1
                xtiles[idx] = X

            pcnt = [0, 0]   # per-block piece-pair counter (H pattern)
            c2cnt = [0]     # copy2 pattern counter

            def tile_gen(tidx):
                """Generator emitting one tile's IR; yields between stages
                so two tiles can be interleaved. Each parity chain has its
                own PSUM pool so the chains never lockstep on the ring."""
                n, cb = tiles[tidx]
                ppool = ppools[tidx % 2]
                wvc = wvt[cb]
                needL, needR, needVm, needVp = flags[cb]
                cs = slice(cb * P, (cb + 1) * P)
                if tidx + 4 < NT:
                    issue_load(tidx + 4)
                X = xtiles.pop(tidx)

                A = apool.tile([P, ALEN], f16)
                nc.gpsimd.memset(A[:, 0:2].bitcast(f32), 0.0)
                nc.gpsimd.memset(A[:, HW:HW + 2].bitcast(f32), 0.0)
                OUT = opool.tile([P, HW], f16)

                vtaps = [t for t, need in enumerate(
                    (needVm, True, needVp)) if need]
                htaps = [t for t, need in enumerate(
                    (needL, True, needR)) if need]

                def emit_v(p0, PZ):
                    PSA = ppool.tile([P, PSLEN], f32, tag="ps")
                    for c0 in range(0, PZ, CHUNK):
                        cn = min(CHUNK, PZ - c0)
                        for i, tap in enumerate(vtaps):
                            o = W + p0 + c0 + (tap - 1) * W
                            nc.tensor.matmul(
                                PSA[:, c0:c0 + cn],
                                wdt[cb][:, tap * P:(tap + 1) * P],
                                X[:, o:o + cn],
                                start=(i == 0), stop=(i == len(vtaps) - 1))
                    nc.scalar.activation(A[:, 1 + p0:1 + p0 + PZ],
                                         PSA[:, 0:PZ], act_copy)

                def emit_h_pe(p0, PZ, rr0, rr1):
                    PSB = ppool.tile([P, PSLEN], f32, tag="ps")
                    for c0 in range(0, PZ, CHUNK):
                        cn = min(CHUNK, PZ - c0)
                        for i, tap in enumerate(htaps):
                            o = p0 + c0 + tap  # A[i + tap], taps 0/1/2
                            nc.tensor.matmul(
                                PSB[:, c0:c0 + cn],
                                wdt[cb][:, (3 + tap) * P:(4 + tap) * P],
                                A[:, o:o + cn],
                                start=(i == 0), stop=(i == len(htaps) - 1))
                    if C2PAT[c2cnt[0] % len(C2PAT)] == 's':
                        nc.scalar.activation(OUT[:, p0:p0 + PZ],
                                             PSB[:, 0:PZ], act_copy)
                    else:
                        nc.vector.tensor_copy(OUT[:, p0:p0 + PZ],
                                              PSB[:, 0:PZ])
                    c2cnt[0] += 1
                    # row-wrap fixups for the flat +-1 taps on this piece
                    o2 = OUT[:, 0:HW].rearrange("p (h w) -> p h w", w=W)
                    if needL:
                        am = A[:, 0:HW].rearrange(
                            "p (h w) -> p h w", w=W)[:, rr0:rr1, 0]
                        nc.vector.scalar_tensor_tensor(
                            o2[:, rr0:rr1, 0], am, wvc[:, 3:4],
                            o2[:, rr0:rr1, 0], op0=mult, op1=add)
                    if needR:
                        ap1 = A[:, 2:2 + HW].rearrange(
                            "p (h w) -> p h w", w=W)[:, rr0:rr1, W - 1]
                        nc.vector.scalar_tensor_tensor(
                            o2[:, rr0:rr1, W - 1], ap1, wvc[:, 4:5],
                            o2[:, rr0:rr1, W - 1], op0=mult, op1=add)

                def emit_h_dve(p0, PZ):
                    # center: OUT = wh_0 * vt
                    nc.vector.tensor_scalar_mul(
                        OUT[:, p0:p0 + PZ], A[:, 1 + p0:1 + p0 + PZ],
                        wvc[:, 1:2])
                    # sides with 2D APs excluding the wrapped column
                    o2 = OUT[:, p0:p0 + PZ].rearrange("p (h w) -> p h w", w=W)
                    v2 = A[:, 1 + p0:1 + p0 + PZ].rearrange(
                        "p (h w) -> p h w", w=W)
                    if needL:  # out[h, 1:] += wh_m1 * vt[h, :-1]
                        nc.vector.scalar_tensor_tensor(
                            o2[:, :, 1:W], v2[:, :, 0:W - 1], wvc[:, 0:1],
                            o2[:, :, 1:W], op0=mult, op1=add)
                    if needR:  # out[h, :-1] += wh_p1 * vt[h, 1:]
                        nc.vector.scalar_tensor_tensor(
                            o2[:, :, 0:W - 1], v2[:, :, 1:W], wvc[:, 2:3],
                            o2[:, :, 0:W - 1], op0=mult, op1=add)

                tb = bounds(tidx)
                pieces = list(zip(tb[:-1], tb[1:]))
                # pair up pieces for the H stage + slab stores
                pairs = [pieces[i:i + 2] for i in range(0, len(pieces), 2)]
                h_left = [len(pr) for pr in pairs]  # pieces awaiting H
                stored = [False] * len(pairs)
                pend_pe = []  # (pair_idx, p0, PZ, rr0, rr1) awaiting copy1

                def maybe_store(j):
                    # store the pair's row slab as soon as its H is complete.
                    # Early tiles store via SWDGE; late tiles via the sync
                    # HWDGE queue, which is idle once all loads are issued
                    # (keeps the SWDGE queue from backlogging into a tail).
                    if h_left[j] == 0 and not stored[j]:
                        stored[j] = True
                        s0, s1 = pairs[j][0][0], pairs[j][-1][1]
                        eng = nc.gpsimd if tidx < NT - 4 else nc.sync
                        eng.dma_start(
                            ys[n, cs, s0:s1, :],
                            OUT[:, s0 * W:s1 * W].rearrange(
                                "p (h w) -> p h w", w=W))

                for j, pair in enumerate(pairs):
                    for rr0, rr1 in pair:
                        emit_v(rr0 * W, (rr1 - rr0) * W)
                        # flush pending PE-H (its +1 tap needed this copy1)
                        for pj, *pp in pend_pe:
                            emit_h_pe(*pp)
                            h_left[pj] -= 1
                            maybe_store(pj)
                        pend_pe = []
                        yield
                    on_pe = HPAT[cb][pcnt[cb] % len(HPAT[cb])] == 'p'
                    pcnt[cb] += 1
                    if on_pe:
                        for rr0, rr1 in pair:
                            pend_pe.append(
                                (j, rr0 * W, (rr1 - rr0) * W, rr0, rr1))
                        # all but the last piece of the pair can emit now
                        for pj, *pp in pend_pe[:-1]:
                            emit_h_pe(*pp)
                            h_left[pj] -= 1
                        pend_pe = pend_pe[-1:]
                    else:
                        rr0 = pair[0][0]
                        rr1 = pair[-1][1]
                        emit_h_dve(rr0 * W, (rr1 - rr0) * W)
                        h_left[j] = 0
                        maybe_store(j)
                    yield

                for pj, *pp in pend_pe:  # last piece: A[HW+1] guard is zero
                    emit_h_pe(*pp)
                    h_left[pj] -= 1
                    maybe_store(pj)

            # wd0 then X0's (finely cut) segments first: the first matmul
            # needs only those; wv0 and the rest follow
            load_wd(0)
            issue_load(0)
            load_wv(0)
            load_wd(1)
            issue_load(1)
            load_wv(1)
            issue_load(2)
            issue_load(3)

            # drive two tile generators interleaved
            from collections import deque
            active = deque([tile_gen(0), tile_gen(1)])
            next_tile = 2
            while active:
                g = active.popleft()
                try:
                    next(g)
                    active.append(g)
                except StopIteration:
                    if next_tile < NT:
                        active.append(tile_gen(next_tile))
                        next_tile += 1
    nc.finalize()
    return nc


def _tap_weights(shift):
    """Per-channel 3-tap weights over offsets {-1,0,1} for shift in [-1,1)."""
    f = np.floor(shift)
    t = (shift - f).astype(np.float32)
    assert np.all((f == -1) | (f == 0)), "shift outside [-1,1) unsupported"
    w_m1 = np.where(f == -1, 1 - t, 0).astype(np.float32)
    w_0 = np.where(f == -1, t, 1 - t).astype(np.float32)
    w_p1 = np.where(f == 0, t, 0).astype(np.float32)
    return w_m1, w_0, w_p1


def _host_prep(sp):
    """Channel sort by W-shift side + weight tensors (sorted order)."""
    beta_side = (np.floor(sp[:, 1]) == 0).astype(np.int32)  # 0=left, 1=right
    perm = np.argsort(beta_side, kind="stable")
    sps = sp[perm]
    wh_m1, wh_0, wh_p1 = _tap_weights(sps[:, 1])  # beta: W shift
    wv_m1, wv_0, wv_p1 = _tap_weights(sps[:, 0])  # alpha: H shift

    flags = []
    for cb in range(CB):
        cs = slice(cb * P, (cb + 1) * P)
        flags.append((bool(wh_m1[cs].any()), bool(wh_p1[cs].any()),
                      bool(wv_m1[cs].any()), bool(wv_p1[cs].any())))

    wd = np.zeros((CB, 6, P, P), np.float32)
    for cb in range(CB):
        cs = slice(cb * P, (cb + 1) * P)
        for t, w in enumerate((wv_m1, wv_0, wv_p1, wh_m1, wh_0, wh_p1)):
            wd[cb, t] = np.diag(w[cs])
    wd = wd.transpose(0, 2, 1, 3).reshape(CB, P, 6 * P).astype(np.float16)
    wvv = np.stack([wh_m1, wh_0, wh_p1, -wh_m1, -wh_p1], axis=1)
    wvv = np.ascontiguousarray(wvv.astype(np.float32).reshape(CB, P, 5))
    return perm, tuple(flags), np.ascontiguousarray(wd), wvv


def _install_trace_shim():
    """Dev-only: register the NTFF profile hook this container's antenv lacks,
    and stub out the artifact upload (zero-egress container)."""
    import sys
    import types

    try:
        from antenv.axon_hooks import get_axon_ntff_profile_hook  # noqa: F401
    except ImportError:
        from trn_agent_boot.trn_boot import _ntff_profile_via_ctypes

        hook = _ntff_profile_via_ctypes("/opt/axon/libaxon_pjrt.so")
        mod = types.ModuleType("antenv.axon_hooks")
        mod.get_axon_ntff_profile_hook = lambda: hook
        mod.set_axon_ntff_profile_hook = lambda h: None
        import antenv

        sys.modules["antenv.axon_hooks"] = mod
        antenv.axon_hooks = mod

    import concourse.bass_utils as bu

    bu.upload_artifacts = lambda tmpdir: tmpdir


def kernel(x, shift_param):
    from concourse.bass_utils import run_bass_kernel_spmd

    x = np.asarray(x)
    sp = np.asarray(shift_param, dtype=np.float32)
    assert x.shape == (N, C, H, W)

    perm, flags, wd, wv = _host_prep(sp)
    x16 = np.ascontiguousarray(x[:, perm].astype(np.float16))

    key = ("nc", flags)
    if key not in _CACHE:
        _CACHE[key] = _build_nc(flags)
    nc = _CACHE[key]

    in_maps = [{"xs": x16[i * NSH:(i + 1) * NSH], "wd": wd, "wv": wv}
               for i in range(NCORES)]
    trace = os.environ.get("ASL_TRACE") == "1"
    if trace:
        _install_trace_shim()
    res = run_bass_kernel_spmd(nc, in_maps, list(range(NCORES)), trace=trace)
    if trace:
        print(f"HW exec time: {res.exec_time_ns} ns")
        _CACHE["last_result"] = res
    ys = np.concatenate([r["ys"] for r in res.results], axis=0)
    out = np.empty((N, C, H, W), np.float32)
    out[:, perm] = ys.astype(np.float32)
    return out
